# revision 3
# baseline (speedup 1.0000x reference)
"""GumbelSoftmaxQuantizationFM kernel for 8 Trainium2 NeuronCores.

Strategy:
- Host: compute gumbel-softmax probs [26,7] (exact 0/1 mask structure), fuse
  tables: joint (k1,k2) codebooks for big fields 0-6, per-k scaled codebooks
  k3-k6 for fields 0-6, fully-mixed rows (emb+lin+sumsq) for fields 7-25.
- Device (batch-sharded 512/core): row gathers via indirect DMA
  (one offset per partition per instruction = 128 rows/instr, the only
  HW-reliable pattern), then DVE reductions for the FM.
"""
import numpy as np

ACTION = np.array([1, 64, 128, 256, 512, 1024, 2048])
FIELD_DIMS = np.array([1000000, 500000, 250000, 100000, 100000, 50000, 50000,
                       10000, 10000, 5000, 5000, 1000, 1000, 500, 500, 200,
                       200, 100, 100, 50, 50, 20, 20, 10, 10, 4])
OFFSETS = np.concatenate([[0], np.cumsum(FIELD_DIMS)])[:-1].astype(np.int64)
F, A, D, BATCH, NCORES = 26, 7, 16, 4096, 8
BC = BATCH // NCORES  # 512 per core

# actions active per field (prefix 1..KF[f]); 0 => only action0 (emb)
def _kf():
    kf = np.zeros(F, np.int64)
    for i in range(F):
        k = 0
        for a in range(1, A):
            if ACTION[a] * 2.5 > FIELD_DIMS[i]:
                break
            k = a
        kf[i] = k
    return kf
KF = _kf()  # [6]*9, 5,5, 3,3, 2,2, 1,1, 0*9

BIG = list(range(0, 7))      # fields with k1..k6, stay 2-level on device
MIXF = list(range(7, 17))    # fields fused into mixed tables (vocab<=10000)
SMALLF = list(range(17, 26)) # action-0 fields (emb)
MIX_SIZES = [int(FIELD_DIMS[f]) for f in MIXF]
MIX_OFF = np.concatenate([[0], np.cumsum(MIX_SIZES)])[:-1]
SMALL_SIZES = [int(FIELD_DIMS[f]) for f in SMALLF]
SMALL_OFF = np.concatenate([[0], np.cumsum(SMALL_SIZES)])[:-1]

_NC_CACHE = {}


def _probs(arch_params, gumbel):
    prior = np.full((F, A), -100000.0, dtype=np.float32)
    for i in range(F):
        if FIELD_DIMS[i] < 150:
            prior[i, 0] = 1.0
        for k in range(1, A):
            if ACTION[k] * 2.5 > FIELD_DIMS[i]:
                break
            prior[i, k] = 1.0
    logits = np.where(prior > 0, arch_params.astype(np.float32),
                      np.float32(-1e9))
    z = logits + gumbel.astype(np.float32)
    z = z - z.max(axis=1, keepdims=True)
    ez = np.exp(z)
    return (ez / ez.sum(axis=1, keepdims=True)).astype(np.float32)


def _build_nc():
    import concourse.bass as bass
    import concourse.bacc as bacc
    import concourse.mybir as mybir
    from concourse.tile import TileContext

    n12, nK, nM, nS = 7 * BC, 28 * BC, 10 * BC, 9 * BC  # idx counts / core
    C12, CK, CM, CS = n12 // 128, nK // 128, nM // 128, nS // 128

    nc = bacc.Bacc("TRN2", target_bir_lowering=False, debug=False)
    T12 = nc.dram_tensor("T12", [7 * 8192, 16], mybir.dt.float32, kind="ExternalInput")
    TK = nc.dram_tensor("TK", [7 * (256 + 512 + 1024 + 2048), 16], mybir.dt.float32, kind="ExternalInput")
    TM = nc.dram_tensor("TM", [int(sum(MIX_SIZES)), 18], mybir.dt.float32, kind="ExternalInput")
    TS = nc.dram_tensor("TS", [int(sum(SMALL_SIZES)), 18], mybir.dt.float32, kind="ExternalInput")
    i12 = nc.dram_tensor("i12", [128, C12], mybir.dt.int32, kind="ExternalInput")
    iK = nc.dram_tensor("iK", [128, CK], mybir.dt.int32, kind="ExternalInput")
    iM = nc.dram_tensor("iM", [128, CM], mybir.dt.int32, kind="ExternalInput")
    iS = nc.dram_tensor("iS", [128, CS], mybir.dt.int32, kind="ExternalInput")
    out = nc.dram_tensor("out", [128, 4], mybir.dt.float32, kind="ExternalOutput")

    with TileContext(nc) as tc:
        with tc.tile_pool(name="cst", bufs=1) as cp, \
             tc.tile_pool(name="wrk", bufs=2) as wp:
            i12_t = cp.tile([128, C12], mybir.dt.int32)
            iK_t = cp.tile([128, CK], mybir.dt.int32)
            iM_t = cp.tile([128, CM], mybir.dt.int32)
            iS_t = cp.tile([128, CS], mybir.dt.int32)
            nc.sync.dma_start(i12_t[:], i12[:])
            nc.sync.dma_start(iK_t[:], iK[:])
            nc.sync.dma_start(iM_t[:], iM[:])
            nc.sync.dma_start(iS_t[:], iS[:])

            d12 = cp.tile([128, C12 * 16], mybir.dt.float32)
            dK = cp.tile([128, CK * 16], mybir.dt.float32)
            dM = cp.tile([128, CM * 18], mybir.dt.float32)
            dS = cp.tile([128, CS * 18], mybir.dt.float32)
            out_sb = cp.tile([128, 4], mybir.dt.float32)

            def gather(dst, dw, tbl, it, C):
                dv = dst[:].rearrange("p (c e) -> p c e", c=C, e=dw)
                for c in range(C):
                    nc.gpsimd.indirect_dma_start(
                        out=dv[:, c, :], out_offset=None, in_=tbl[:],
                        in_offset=bass.IndirectOffsetOnAxis(
                            ap=it[:, c:c + 1], axis=0))

            gather(d12, 16, T12, i12_t, C12)
            gather(dK, 16, TK, iK_t, CK)
            gather(dM, 18, TM, iM_t, CM)
            gather(dS, 18, TS, iS_t, CS)

            r12 = d12[:].rearrange("p (q t e) -> p q t e", q=7, t=4, e=16)
            rK = dK[:].rearrange("p (k q t e) -> p k q t e", k=4, q=7, t=4, e=16)
            rM = dM[:].rearrange("p (q t e) -> p q t e", q=10, t=4, e=18)
            rS = dS[:].rearrange("p (q t e) -> p q t e", q=9, t=4, e=18)

            for t in range(4):
                e = wp.tile([128, 7 * 16], mybir.dt.float32, tag="e")
                ev = e[:].rearrange("p (f d) -> p f d", f=7, d=16)
                nc.vector.tensor_add(ev[:, :, :], r12[:, :, t, :], rK[:, 0, :, t, :])
                nc.vector.tensor_add(ev[:, :, :], ev[:, :, :], rK[:, 1, :, t, :])
                nc.vector.tensor_add(ev[:, :, :], ev[:, :, :], rK[:, 2, :, t, :])
                nc.vector.tensor_add(ev[:, :, :], ev[:, :, :], rK[:, 3, :, t, :])

                import concourse.mybir as mb
                s7 = wp.tile([128, 16], mybir.dt.float32, tag="s7")
                nc.vector.tensor_reduce(
                    out=s7[:], in_=e[:].rearrange("p (f d) -> p d f", f=7, d=16),
                    axis=mb.AxisListType.X, op=mb.AluOpType.add)
                gM = wp.tile([128, 18], mybir.dt.float32, tag="gM")
                nc.vector.tensor_reduce(
                    out=gM[:], in_=rM[:, :, t, :].rearrange("p q e -> p e q"),
                    axis=mb.AxisListType.X, op=mb.AluOpType.add)
                gS = wp.tile([128, 18], mybir.dt.float32, tag="gS")
                nc.vector.tensor_reduce(
                    out=gS[:], in_=rS[:, :, t, :].rearrange("p q e -> p e q"),
                    axis=mb.AxisListType.X, op=mb.AluOpType.add)

                s = wp.tile([128, 16], mybir.dt.float32, tag="s")
                nc.vector.tensor_add(s[:], s7[:], gM[:, 0:16])
                nc.vector.tensor_add(s[:], s[:], gS[:, 0:16])

                e2 = wp.tile([128, 7 * 16], mybir.dt.float32, tag="e2")
                nc.vector.tensor_mul(e2[:], e[:], e[:])
                sq7 = wp.tile([128, 1], mybir.dt.float32, tag="sq7")
                nc.vector.tensor_reduce(out=sq7[:], in_=e2[:],
                                        axis=mb.AxisListType.X, op=mb.AluOpType.add)
                s2 = wp.tile([128, 16], mybir.dt.float32, tag="s2")
                nc.vector.tensor_mul(s2[:], s[:], s[:])
                s2r = wp.tile([128, 1], mybir.dt.float32, tag="s2r")
                nc.vector.tensor_reduce(out=s2r[:], in_=s2[:],
                                        axis=mb.AxisListType.X, op=mb.AluOpType.add)

                sq = wp.tile([128, 1], mybir.dt.float32, tag="sq")
                nc.vector.tensor_add(sq[:], sq7[:], gM[:, 17:18])
                nc.vector.tensor_add(sq[:], sq[:], gS[:, 17:18])
                lin = wp.tile([128, 1], mybir.dt.float32, tag="lin")
                nc.vector.tensor_add(lin[:], gM[:, 16:17], gS[:, 16:17])

                fm = wp.tile([128, 1], mybir.dt.float32, tag="fm")
                nc.vector.tensor_sub(fm[:], s2r[:], sq[:])
                nc.scalar.mul(fm[:], fm[:], 0.5)
                nc.vector.tensor_add(out_sb[:, t:t + 1], fm[:], lin[:])

            nc.sync.dma_start(out[:], out_sb[:])

    nc.finalize()
    return nc


def kernel(x, emb_table, lin_w, lin_bias, codebooks, assignments,
           arch_params, gumbel):
    x = np.asarray(x); emb_table = np.asarray(emb_table)
    lin_w = np.asarray(lin_w); lin_bias = np.asarray(lin_bias)
    codebooks = np.asarray(codebooks); assignments = np.asarray(assignments)
    w = _probs(np.asarray(arch_params), np.asarray(gumbel))

    # ---- tables (fp32) ----
    # T12: joint (k1,k2) for fields 0-6: row f*8192 + c1*128 + c2
    T12 = (w[0:7, 1][:, None, None, None] * codebooks[0, 0:7, 0:64, None, :]
           + w[0:7, 2][:, None, None, None] * codebooks[1, 0:7, None, 0:128, :]
           ).reshape(7 * 8192, 16).astype(np.float32)
    # TK: k=3..6 scaled slices for fields 0-6, concatenated k-major
    tk_parts = []
    for k in range(3, 7):
        Ak = int(ACTION[k])
        tk_parts.append((w[0:7, k][:, None, None]
                         * codebooks[k - 1, 0:7, 0:Ak, :]).reshape(-1, 16))
    TK = np.concatenate(tk_parts, 0).astype(np.float32)
    TK_OFF = np.concatenate([[0], np.cumsum(
        [7 * int(ACTION[k]) for k in range(3, 7)])])[:-1]

    # TM: fully mixed rows for fields 7-16: [mix(16) | lin | sumsq]
    TM = np.zeros((int(sum(MIX_SIZES)), 18), np.float32)
    for j, f in enumerate(MIXF):
        v = int(FIELD_DIMS[f]); off = int(OFFSETS[f])
        m = np.zeros((v, 16), np.float32)
        for k in range(1, KF[f] + 1):
            m += w[f, k] * codebooks[k - 1, f, assignments[k - 1, off:off + v]]
        sl = slice(int(MIX_OFF[j]), int(MIX_OFF[j]) + v)
        TM[sl, 0:16] = m
        TM[sl, 16] = lin_w[off:off + v, 0]
        TM[sl, 17] = (m * m).sum(1)
    # TS: fields 17-25 (emb * probs[f,0], lin, sumsq)
    TS = np.zeros((int(sum(SMALL_SIZES)), 18), np.float32)
    for j, f in enumerate(SMALLF):
        v = int(FIELD_DIMS[f]); off = int(OFFSETS[f])
        m = (w[f, 0] * emb_table[off:off + v]).astype(np.float32)
        sl = slice(int(SMALL_OFF[j]), int(SMALL_OFF[j]) + v)
        TS[sl, 0:16] = m
        TS[sl, 16] = lin_w[off:off + v, 0]
        TS[sl, 17] = (m * m).sum(1)

    # ---- indices ----
    gid_big = x[:, 0:7].astype(np.int64) + OFFSETS[None, 0:7]
    lin_big = lin_w[gid_big, 0].astype(np.float32).sum(1)  # [B]
    codes = {k: assignments[k - 1, gid_big].astype(np.int64)
             for k in range(1, 7)}  # [B,7]
    idx12 = (np.arange(7)[None, :] * 8192 + codes[1] * 128 + codes[2])  # [B,7]
    idxK = np.concatenate(
        [TK_OFF[k - 3] + np.arange(7)[None, :] * int(ACTION[k]) + codes[k]
         for k in range(3, 7)], axis=1)  # [B,28]
    idxM = (MIX_OFF[None, :] + x[:, 7:17].astype(np.int64))  # [B,10]
    idxS = (SMALL_OFF[None, :] + x[:, 17:26].astype(np.int64))  # [B,9]

    def core_idx(a, c):  # [B,Q] -> [128, Q*4] int32 (i = q*512+b stream)
        loc = a[c * BC:(c + 1) * BC]          # [512, Q]
        iv = loc.T.reshape(-1)                # i = q*512 + b
        return np.ascontiguousarray(iv.reshape(-1, 128).T).astype(np.int32)

    key = "nc"
    if key not in _NC_CACHE:
        _NC_CACHE[key] = _build_nc()
    nc = _NC_CACHE[key]

    in_maps = []
    for c in range(NCORES):
        in_maps.append({
            "T12": T12, "TK": TK, "TM": TM, "TS": TS,
            "i12": core_idx(idx12, c), "iK": core_idx(idxK, c),
            "iM": core_idx(idxM, c), "iS": core_idx(idxS, c)})

    from concourse.bass_utils import run_bass_kernel_spmd
    res = run_bass_kernel_spmd(nc, in_maps, core_ids=list(range(NCORES)))

    out = np.zeros(BATCH, np.float32)
    for c in range(NCORES):
        o = res.results[c]["out"]  # [128, 4]: b = t*128+p
        out[c * BC:(c + 1) * BC] = o.T.reshape(-1)
    return out + lin_big + np.float32(lin_bias[0])
